# revision 18
# baseline (speedup 1.0000x reference)
"""Trainium2 Bass kernel: dense multi-head dot-product attention.

Problem: x [4, 2048, 1024], W_Q/W_K/W_V [16, 1024, 64] ->
         out [4, 2048, 1024] (heads concatenated on the feature dim).

Sharding: 8 cores = 4 batches x 2 head-groups (8 heads each).
Per core, everything is computed in "transposed" layouts so that no
on-chip transpose of the big attention matrix is ever needed:
  - x^T [1024, 2048] (n on partitions) per batch, loaded via HWDGE
  - projections (W stationary): Q^T/K^T/V^T [heads*64, 2048]
  - scores S^T[k, m] = sum_d K^T[d,k] Q^T[d,m]  (k on partitions)
  - P^T = exp(S^T/8)  (elementwise, ScalarE, PSUM->SBUF)
  - O^T[d, m] = sum_k Vaug[k, d] P^T[k, m] with Vaug = [V | ones | 0...],
    so row 64 of the accumulator is the softmax denominator.
  - normalize: recip(row64) broadcast over partitions (GpSimd), DVE mul
  - output O^T [512, 2048] per core; host transposes when gathering.
Softmax skips the max-subtraction: |S/8| < ~12 here, exp is safe in fp32
and softmax is shift-invariant, so the result is mathematically identical.

Performance structure: every matmul is a uniform (128,128)-tile config.
HW shows the PE pays ~100ns on each tile-config switch (64-row score
tiles alternating with 65-col PV tiles in the old layout), so:
  - score stationary is ktz_h [128, 128]: the head's K^T block in its
    own 64 d-rows, zeros in the other head's rows. Moving data is the
    full 2-head Q^T [128, 512]; the zero rows contribute nothing.
  - PV stationary is vaug[:, kc, hp] [128, 128]: 64 V cols + ones col
    + 63 zero cols; ot PSUM rows 65..127 accumulate zeros, never read.
Matmul operands are float32r end-to-end (fp32 bits; the PE rounds
internally, ~1e-4 rel err, 1 cyc/row at free-dim 512). DRAM inputs are
declared float32r so input DMAs are dtype-matched and ride hardware
DGE (sync+scalar queues) instead of the software-DGE cast path: first
m-quarter of all x^T chunks first so projections start at ~2MB loaded.
V-transposes run in f32r (1.5 cyc/row vs 2.0 for f32). Zero/one
constant regions are written via tensor_copy from f32 const tiles
(walrus accepts cast-copies as f32r-rounded producers; memset straight
into f32r it does not).

PSUM: shared tag (S^T chunks / proj accum / V-transposes)
                   3 bufs x [128,1024] fp32 = 6 banks
      ot (O^T)     1 buf  x [128,1024] fp32 = 2 banks
"""

from contextlib import ExitStack

import numpy as np

import concourse.bass as bass  # noqa: F401  (bass types via bacc)
import concourse.tile as tile
from concourse import bacc, mybir
from concourse import bass_utils
from concourse.masks import make_identity

F32 = mybir.dt.float32
F32R = mybir.dt.float32r

B, M, N, H, D = 4, 2048, 1024, 16, 64
HPC = 8          # heads per core
NCORES = 8
NCH = 8          # d_model / 128 chunks
KC = 16          # key chunks of 128
SCALE = 0.125    # 1/sqrt(64)
MH = 1024        # m-half width


def build_nc():
    nc = bacc.Bacc(
        "TRN2", target_bir_lowering=False, debug=False, enable_asserts=False
    )
    xt_d = nc.dram_tensor("xt", [N, M], F32R, kind="ExternalInput")
    wq_d = nc.dram_tensor("wq", [4, N, 128], F32R, kind="ExternalInput")
    wk_d = nc.dram_tensor("wk", [4, N, 128], F32R, kind="ExternalInput")
    wv_d = nc.dram_tensor("wv", [4, N, 128], F32R, kind="ExternalInput")
    o_d = nc.dram_tensor("ot", [HPC * D, M], F32, kind="ExternalOutput")

    with tile.TileContext(nc) as tc, ExitStack() as ctx:
        const_pool = ctx.enter_context(tc.tile_pool(name="constp", bufs=1))
        xt_pool = ctx.enter_context(tc.tile_pool(name="xtp", bufs=NCH))
        w_pool = ctx.enter_context(tc.tile_pool(name="wp", bufs=4))
        q_pool = ctx.enter_context(tc.tile_pool(name="qp", bufs=2))
        ktz_pool = ctx.enter_context(tc.tile_pool(name="ktzp", bufs=2))
        vt_pool = ctx.enter_context(tc.tile_pool(name="vtp", bufs=1))
        vaug_pool = ctx.enter_context(tc.tile_pool(name="vaugp", bufs=2))
        pt_pool = ctx.enter_context(tc.tile_pool(name="ptp", bufs=4))
        out_pool = ctx.enter_context(tc.tile_pool(name="outp", bufs=3))
        small_pool = ctx.enter_context(tc.tile_pool(name="smallp", bufs=3))
        st_pool = ctx.enter_context(tc.tile_pool(name="stp", bufs=3, space="PSUM"))
        ot_pool = ctx.enter_context(tc.tile_pool(name="otp", bufs=1, space="PSUM"))

        identf = const_pool.tile([128, 128], F32, name="identf")
        make_identity(nc, identf[:])
        # f32r copy of the identity so V-transposes run at 1.5 cyc/row
        # (f32 transposes are 2.0); affine_select can't target f32r, so
        # build in f32 and cast-copy.
        ident = const_pool.tile([128, 128], F32R, name="ident")
        nc.vector.tensor_copy(ident[:], identf[:])
        # onespad: col 0 = 1.0 (softmax-denominator column), cols 1..63 = 0
        onespad = const_pool.tile([128, 64], F32, name="onespad")
        nc.gpsimd.memset(onespad[:], 0.0)
        nc.gpsimd.memset(onespad[:, 0:1], 1.0)
        zeros64 = const_pool.tile([64, 512], F32, name="zeros64")
        nc.gpsimd.memset(zeros64[:], 0.0)
        ones16 = const_pool.tile([128, 16, 1], F32, name="ones16")
        nc.gpsimd.memset(ones16[:], 1.0)

        # ---- resident x^T tiles, loaded via HWDGE on sync+scalar queues.
        # First m-quarter of all chunks first so projections start early.
        xts = []
        for c in range(NCH):
            xtile = xt_pool.tile([128, M], F32R, name=f"xt{c}", tag="xtile")
            eng = nc.sync if c % 2 == 0 else nc.scalar
            eng.dma_start(
                xtile[:, 0:512], xt_d.ap()[c * 128:(c + 1) * 128, 0:512]
            )
            xts.append(xtile)
        for q in range(1, 4):
            for c in range(NCH):
                eng = nc.sync if c % 2 == 0 else nc.scalar
                eng.dma_start(
                    xts[c][:, q * 512:(q + 1) * 512],
                    xt_d.ap()[c * 128:(c + 1) * 128, q * 512:(q + 1) * 512],
                )

        for p in range(4):  # head pairs
            # ---- projections: ps[h%2*64+d, m] for the two heads of pair p
            wts = {}
            for nm, wd in (("q", wq_d), ("k", wk_d), ("v", wv_d)):
                wt = w_pool.tile([128, NCH, 128], F32R, name=f"wt_{nm}", tag="wt")
                nc.gpsimd.dma_start(
                    wt[:], wd.ap()[p].rearrange("(c p) d -> p c d", p=128)
                )
                wts[nm] = wt

            qt = q_pool.tile([128, M], F32R, name="qt", tag="qt")
            ktz0 = ktz_pool.tile([128, M], F32R, name="ktz0", tag="ktz0")
            ktz1 = ktz_pool.tile([128, M], F32R, name="ktz1", tag="ktz1")
            vt = vt_pool.tile([128, M], F32R, name="vt", tag="vt")
            # zero the unused head's rows so score matmuls can contract
            # over the full 128 partitions of the 2-head Q^T tile
            # (tensor_copy from an f32 const: walrus accepts cast-copies as
            # f32r-rounded producers; memset straight into f32r it does not)
            for mq in range(4):
                qsl = slice(mq * 512, (mq + 1) * 512)
                nc.vector.tensor_copy(ktz0[64:128, qsl], zeros64[:])
                nc.vector.tensor_copy(ktz1[0:64, qsl], zeros64[:])

            for nm in ("q", "k", "v"):
                wt = wts[nm]
                for mh in range(2):
                    msl = slice(mh * MH, (mh + 1) * MH)
                    ps = st_pool.tile([128, MH], F32, name="ps_prj", tag="st")
                    for c in range(NCH):
                        for mc in range(2):
                            nc.tensor.matmul(
                                ps[:, mc * 512:(mc + 1) * 512],
                                lhsT=wt[:, c, :],
                                rhs=xts[c][
                                    :,
                                    mh * MH + mc * 512: mh * MH + (mc + 1) * 512,
                                ],
                                start=(c == 0),
                                stop=(c == NCH - 1),
                                skip_group_check=True,
                            )
                    if nm == "q":
                        nc.vector.tensor_copy(qt[:, msl], ps[:])
                    elif nm == "k":
                        nc.vector.tensor_copy(ktz0[0:64, msl], ps[0:64, :])
                        nc.vector.tensor_copy(ktz1[64:128, msl], ps[64:128, :])
                    else:
                        nc.vector.tensor_copy(vt[:, msl], ps[:])

            # ---- vaug[k, kc, 0:193]: [V0|ones|V1|ones|zeros63], head
            # stationaries overlap at stride 65 (h0 = cols 0:128 with V1
            # data as harmless junk in 65:128 — ot rows 65..127 unread;
            # h1 = cols 65:193 with a zero tail). 12.1KB/partition vs 16.
            vaug = vaug_pool.tile([128, KC, 193], F32R, name="vaug", tag="vaug")
            for hp in range(2):
                nc.vector.tensor_copy(
                    vaug[:, :, 65 * hp + 64:65 * hp + 65], ones16[:]
                )
            for kc in range(KC):
                nc.vector.tensor_copy(vaug[:, kc, 130:193], onespad[:, 1:64])
            for kc in range(KC):
                trp = st_pool.tile([128, 128], F32R, name="trp", tag="st")
                nc.tensor.transpose(
                    trp[:], vt[:, kc * 128:(kc + 1) * 128], ident[:]
                )
                nc.vector.tensor_copy(
                    vaug[:, kc, 0:130].rearrange(
                        "p (h x) -> p h x", h=2
                    )[:, :, 0:64],
                    trp[:].rearrange("p (h d) -> p h d", h=2),
                )

            # ---- attention per head, split in m-halves of 1024
            for hp in range(2):
                h = 2 * p + hp
                ktz = ktz0 if hp == 0 else ktz1
                for mh in range(2):
                    mbase = mh * MH
                    ot = ot_pool.tile([128, MH], F32, name="ot", tag="ot")
                    for kc in range(KC):
                        st = st_pool.tile([128, MH], F32, name="st", tag="st")
                        for mc in range(2):
                            nc.tensor.matmul(
                                st[:, mc * 512:(mc + 1) * 512],
                                lhsT=ktz[:, kc * 128:(kc + 1) * 128],
                                rhs=qt[
                                    :,
                                    mbase + mc * 512: mbase + (mc + 1) * 512,
                                ],
                                start=True,
                                stop=True,
                            )
                        pt = pt_pool.tile([128, MH], F32R, name="pt", tag="pt")
                        nc.scalar.activation(
                            pt[:], st[:],
                            mybir.ActivationFunctionType.Exp, scale=SCALE,
                        )
                        for mc in range(2):
                            nc.tensor.matmul(
                                ot[:, mc * 512:(mc + 1) * 512],
                                lhsT=vaug[:, kc, 65 * hp:65 * hp + 128],
                                rhs=pt[:, mc * 512:(mc + 1) * 512],
                                start=(kc == 0),
                                stop=(kc == KC - 1),
                                skip_group_check=True,
                            )
                    # ---- normalize rows 0..63 by row 64; free ot ASAP
                    sumsb = small_pool.tile([1, MH], F32, name="sumsb", tag="sm")
                    nc.vector.tensor_copy(sumsb[:], ot[64:65, :])
                    ostage = out_pool.tile([64, MH], F32, name="ostage", tag="o64")
                    nc.vector.tensor_copy(ostage[:], ot[0:64, :])
                    recipb = small_pool.tile([1, MH], F32, name="recipb", tag="sm")
                    scratch = small_pool.tile([1, MH], F32, name="scr", tag="sm")
                    nc.vector.reciprocal_approx_accurate(
                        recipb[:], sumsb[:], scratch[:]
                    )
                    rbc = out_pool.tile([64, MH], F32, name="rbc", tag="o64")
                    nc.gpsimd.partition_broadcast(rbc[:], recipb[:])
                    stage = out_pool.tile([64, MH], F32, name="stage", tag="o64")
                    nc.vector.tensor_mul(stage[:], ostage[:], rbc[:])
                    nc.sync.dma_start(
                        o_d.ap()[h * 64:(h + 1) * 64, mbase:mbase + MH], stage[:]
                    )
    nc.compile()
    return nc


_NC_CACHE = None


def _get_nc():
    global _NC_CACHE
    if _NC_CACHE is None:
        _NC_CACHE = build_nc()
    return _NC_CACHE


def make_in_maps(x, W_Q, W_K, W_V):
    x = np.asarray(x, dtype=np.float32)
    W_Q = np.asarray(W_Q, dtype=np.float32)
    W_K = np.asarray(W_K, dtype=np.float32)
    W_V = np.asarray(W_V, dtype=np.float32)

    def prep_w(W, g):
        blk = W[8 * g:8 * g + 8]  # [8, 1024, 64]
        # pair-major [4, 1024, 128]: col = (head%2)*64 + d
        return np.ascontiguousarray(
            blk.reshape(4, 2, N, D).transpose(0, 2, 1, 3).reshape(4, N, 2 * D)
        )

    in_maps = []
    for c in range(NCORES):
        b, g = divmod(c, 2)
        in_maps.append(
            {
                "xt": np.ascontiguousarray(x[b].T),
                "wq": prep_w(W_Q, g),
                "wk": prep_w(W_K, g),
                "wv": prep_w(W_V, g),
            }
        )
    return in_maps


def gather_out(results):
    out = np.empty((B, M, N), dtype=np.float32)
    for c in range(NCORES):
        b, g = divmod(c, 2)
        out[b, :, 512 * g:512 * (g + 1)] = results[c]["ot"].T
    return out


def run(x, W_Q, W_K, W_V, **spmd_kwargs):
    nc = _get_nc()
    in_maps = make_in_maps(x, W_Q, W_K, W_V)
    res = bass_utils.run_bass_kernel_spmd(
        nc, in_maps, core_ids=list(range(NCORES)), **spmd_kwargs
    )
    return gather_out(res.results), res


def kernel(x, W_Q, W_K, W_V):
    out, _ = run(x, W_Q, W_K, W_V)
    return out
